# revision 6
# baseline (speedup 1.0000x reference)
import os
import numpy as np
from concourse import bass, tile
from concourse import mybir
from concourse.bass_utils import run_bass_kernel_spmd
import bass_rust as _bass_rust

dt = mybir.dt
Alu = mybir.AluOpType
Act = mybir.ActivationFunctionType

N = 4096
F = 512
C = 751
SIDE = 1024
NCORES = 8
RPC = N // NCORES      # 512 rows per core
NT = RPC // 128        # 4 row tiles per core
NEGBIG = np.float32(-1e30)

LAST_EXEC_NS = None


def _build_program():
    nc = bass.Bass()
    xtp_d = [nc.dram_tensor(f"xtp{k}", [128, N + RPC], dt.float32r,
                            kind="ExternalInput") for k in range(4)]
    rk2_d = nc.dram_tensor("rk2", [2, N + RPC], dt.float32, kind="ExternalInput")
    ms_d = nc.dram_tensor("mstrip", [128, 1800], dt.float32, kind="ExternalInput")
    cls_d = nc.dram_tensor("cls", [RPC, C], dt.float32, kind="ExternalInput")
    l2_d = nc.dram_tensor("l2", [RPC, SIDE], dt.float32, kind="ExternalInput")
    l3_d = nc.dram_tensor("l3", [RPC, SIDE], dt.float32, kind="ExternalInput")
    l4_d = nc.dram_tensor("l4", [RPC, SIDE], dt.float32, kind="ExternalInput")
    out_d = nc.dram_tensor("out", [128, 18], dt.float32, kind="ExternalOutput")

    with tile.TileContext(nc) as tc:
        with tc.tile_pool(name="sb", bufs=1) as sb, \
             tc.tile_pool(name="ps", bufs=8, space="PSUM") as ps:
            xtp_t = [sb.tile([128, N + RPC], dt.float32r, name=f"xtp_t{k}")
                     for k in range(4)]
            rk2_t = sb.tile([2, N + RPC], dt.float32)
            ms_t = sb.tile([128, 1800], dt.float32)
            cls_t = [sb.tile([128, C], dt.float32, name=f"cls_t{r}")
                     for r in range(NT)]
            l2_t = [sb.tile([128, SIDE], dt.float32, name=f"l2_t{r}")
                    for r in range(NT)]
            l3_t = [sb.tile([128, SIDE], dt.float32, name=f"l3_t{r}")
                    for r in range(NT)]
            l4_t = [sb.tile([128, SIDE], dt.float32, name=f"l4_t{r}")
                    for r in range(NT)]
            for k in range(4):
                nc.gpsimd.dma_start(xtp_t[k][:], xtp_d[k][:])
            nc.gpsimd.dma_start(rk2_t[:], rk2_d[:])
            nc.gpsimd.dma_start(ms_t[:], ms_d[:])
            for r in range(NT):
                nc.gpsimd.dma_start(cls_t[r][:], cls_d[128 * r:128 * r + 128, :])
                nc.gpsimd.dma_start(l2_t[r][:], l2_d[128 * r:128 * r + 128, :])
                nc.gpsimd.dma_start(l3_t[r][:], l3_d[128 * r:128 * r + 128, :])
                nc.gpsimd.dma_start(l4_t[r][:], l4_d[128 * r:128 * r + 128, :])

            out_t = sb.tile([128, 18], dt.float32)
            a2 = sb.tile([128, NT], dt.float32)
            a3 = sb.tile([128, NT], dt.float32)
            kt = ms_t[:, 1792:1800]

            for r in range(NT):
                lT = slice(N + 128 * r, N + 128 * r + 128)
                cand = sb.tile([128, 64], dt.float32)
                pos8 = sb.tile([128, 8], dt.float32)
                for cb in range(8):
                    cS = slice(512 * cb, 512 * cb + 512)
                    p = ps.tile([128, 512], dt.float32)
                    for k in range(4):
                        nc.tensor.matmul(p[:], xtp_t[k][:, lT], xtp_t[k][:, cS],
                                         start=(k == 0), stop=False)
                    nc.tensor.matmul(p[:], rk2_t[:, lT], rk2_t[:, cS],
                                     start=False, stop=True)
                    if cb == 0:
                        tmpA = sb.tile([128, 512], dt.float32)
                        nc.vector.tensor_tensor(
                            tmpA[:], p[:], ms_t[:, 384 - 128 * r:896 - 128 * r],
                            Alu.add)
                        nc.vector.max(pos8[:], tmpA[:])
                        tmpB = sb.tile([128, 512], dt.float32)
                        nc.vector.tensor_tensor(
                            tmpB[:], p[:], ms_t[:, 1280 - 128 * r:1792 - 128 * r],
                            Alu.add)
                        nc.vector.max(cand[:, 0:8], tmpB[:])
                    else:
                        nc.vector.max(cand[:, 8 * cb:8 * cb + 8], p[:])

                neg8 = sb.tile([128, 8], dt.float32)
                nc.vector.max(neg8[:], cand[:])
                pos8r = sb.tile([128, 8], dt.float32)
                nc.vector.tensor_scalar_add(pos8r[:], pos8[:, 7::-1], 0.0)
                cmp = sb.tile([128, 8], dt.float32)
                nc.vector.tensor_tensor(cmp[:], neg8[:], pos8r[:], Alu.is_gt)
                m_t = sb.tile([128, 1], dt.float32)
                nc.vector.tensor_reduce(m_t[:], cmp[:], mybir.AxisListType.X,
                                        Alu.add)
                clampP = sb.tile([128, 8], dt.float32)
                nc.vector.tensor_scalar(clampP[:], pos8r[:], -1.0, 1e-12,
                                        Alu.mult, Alu.max)
                pP = sb.tile([128, 8], dt.float32)
                nc.scalar.activation(pP[:], clampP[:], Act.Sqrt)
                clampN = sb.tile([128, 8], dt.float32)
                nc.vector.tensor_scalar(clampN[:], neg8[:], -1.0, 1e-12,
                                        Alu.mult, Alu.max)
                nN = sb.tile([128, 8], dt.float32)
                nc.scalar.activation(nN[:], clampN[:], Act.Sqrt)
                n0e = sb.tile([128, 1], dt.float32)
                nc.vector.tensor_scalar_add(n0e[:], nN[:, 0:1], 1e-12)
                rec = sb.tile([128, 1], dt.float32)
                nc.vector.reciprocal(rec[:], n0e[:])
                dlt = sb.tile([128, 8], dt.float32)
                nc.vector.tensor_scalar(dlt[:], nN[:], nN[:, 0:1], -1.0,
                                        Alu.subtract, Alu.mult)
                rat = sb.tile([128, 8], dt.float32)
                nc.vector.tensor_scalar(rat[:], dlt[:], rec[:], 0.0,
                                        Alu.mult, Alu.add)
                E = sb.tile([128, 8], dt.float32)
                nc.scalar.activation(E[:], rat[:], Act.Exp)
                w0 = sb.tile([128, 8], dt.float32)
                nc.vector.tensor_scalar(w0[:], kt, m_t[:], -1.0,
                                        Alu.subtract, Alu.mult)
                ind = sb.tile([128, 8], dt.float32)
                nc.vector.tensor_scalar(ind[:], w0[:], 0.0, 1.0,
                                        Alu.max, Alu.min)
                diff = sb.tile([128, 8], dt.float32)
                nc.vector.tensor_tensor(diff[:], pP[:], nN[:], Alu.subtract)
                t1 = sb.tile([128, 8], dt.float32)
                nc.vector.tensor_tensor(t1[:], E[:], diff[:], Alu.mult)
                t2 = sb.tile([128, 8], dt.float32)
                nc.vector.tensor_tensor(t2[:], t1[:], w0[:], Alu.mult)
                t3 = sb.tile([128, 8], dt.float32)
                nc.vector.tensor_tensor(t3[:], t2[:], ind[:], Alu.mult)
                t5 = sb.tile([128, 8], dt.float32)
                nc.vector.tensor_scalar(t5[:], ind[:], 0.5, 0.0,
                                        Alu.mult, Alu.add)
                l8 = sb.tile([128, 8], dt.float32)
                nc.vector.tensor_tensor(l8[:], t3[:], t5[:], Alu.add)
                nc.vector.tensor_reduce(out_t[:, 4 * r:4 * r + 1], l8[:],
                                        mybir.AxisListType.X, Alu.add)
                nc.vector.tensor_scalar_add(out_t[:, 4 * r + 1:4 * r + 2],
                                            m_t[:], 0.0)

                negmax = sb.tile([128, 1], dt.float32)
                nc.vector.tensor_reduce(negmax[:], cls_t[r][:],
                                        mybir.AxisListType.X, Alu.max,
                                        negate=True)
                scr = sb.tile([128, C], dt.float32)
                se = sb.tile([128, 1], dt.float32)
                nc.scalar.activation(scr[:], cls_t[r][:], Act.Exp,
                                     bias=negmax[:], scale=1.0,
                                     accum_out=se[:])
                lse = sb.tile([128, 1], dt.float32)
                nc.scalar.activation(lse[:], se[:], Act.Ln)
                nc.vector.tensor_tensor(out_t[:, 4 * r + 2:4 * r + 3], lse[:],
                                        negmax[:], Alu.subtract)

                sd2 = sb.tile([128, SIDE], dt.float32)
                nc.vector.tensor_tensor(sd2[:], l4_t[r][:], l2_t[r][:],
                                        Alu.subtract)
                sq2 = sb.tile([128, SIDE], dt.float32)
                nc.scalar.activation(sq2[:], sd2[:], Act.Square,
                                     accum_out=a2[:, r:r + 1])
                sd3 = sb.tile([128, SIDE], dt.float32)
                nc.vector.tensor_tensor(sd3[:], l4_t[r][:], l3_t[r][:],
                                        Alu.subtract)
                sq3 = sb.tile([128, SIDE], dt.float32)
                nc.scalar.activation(sq3[:], sd3[:], Act.Square,
                                     accum_out=a3[:, r:r + 1])

            nc.vector.tensor_reduce(out_t[:, 16:17], a2[:],
                                    mybir.AxisListType.X, Alu.add)
            nc.vector.tensor_reduce(out_t[:, 17:18], a3[:],
                                    mybir.AxisListType.X, Alu.add)
            nc.sync.dma_start(out_d[:], out_t[:])

    _bass_rust.move_matmul_waits_to_ldweights(nc.m)
    _bass_rust.generate_event_semaphores(nc)
    return nc


def _build_mstrip():
    K0 = np.full((128, 128), NEGBIG, np.float32)
    D0 = np.zeros((128, 128), np.float32)
    for b in range(16):
        K0[8 * b:8 * b + 8, 8 * b:8 * b + 8] = 0.0
        D0[8 * b:8 * b + 8, 8 * b:8 * b + 8] = NEGBIG
    kstrip = np.full((128, 896), NEGBIG, np.float32)
    kstrip[:, 384:512] = K0
    drop = np.zeros((128, 896), np.float32)
    drop[:, 384:512] = D0
    kv = np.tile(np.arange(8, dtype=np.float32), (128, 1))
    return np.ascontiguousarray(np.concatenate([kstrip, drop, kv], axis=1))


def _make_in_maps(cls_fea, l2, l3, l4, x):
    sq = (x.astype(np.float64) ** 2).sum(1).astype(np.float32)
    xT = np.ascontiguousarray(x.T)
    mstrip = _build_mstrip()

    in_maps = []
    for c in range(NCORES):
        R0 = RPC * c
        perm = np.concatenate([np.arange(R0, R0 + RPC),
                               np.arange(0, R0),
                               np.arange(R0 + RPC, N)])
        xt_perm = xT[:, perm]
        im = {}
        for k in range(4):
            im[f"xtp{k}"] = np.ascontiguousarray(np.concatenate(
                [xt_perm[128 * k:128 * k + 128, :],
                 2.0 * xt_perm[128 * k:128 * k + 128, 0:RPC]], axis=1))
        sqp = sq[perm]
        rk2 = np.empty((2, N + RPC), np.float32)
        rk2[0, 0:N] = -sqp
        rk2[0, N:] = 1.0
        rk2[1, 0:N] = 1.0
        rk2[1, N:] = -sq[R0:R0 + RPC]
        im["rk2"] = rk2
        im["mstrip"] = mstrip
        im["cls"] = np.ascontiguousarray(cls_fea[R0:R0 + RPC])
        im["l2"] = np.ascontiguousarray(l2[R0:R0 + RPC])
        im["l3"] = np.ascontiguousarray(l3[R0:R0 + RPC])
        im["l4"] = np.ascontiguousarray(l4[R0:R0 + RPC])
        in_maps.append(im)
    return in_maps


def _postprocess(results, cls_fea, x, targets):
    losses = np.empty(N, np.float64)
    ms = np.empty(N, np.float64)
    lse = np.empty(N, np.float64)
    s2 = 0.0
    s3 = 0.0
    for c in range(NCORES):
        o = np.asarray(results[c]["out"], np.float64)
        for r in range(NT):
            rows = slice(RPC * c + 128 * r, RPC * c + 128 * r + 128)
            losses[rows] = o[:, 4 * r]
            ms[rows] = o[:, 4 * r + 1]
            lse[rows] = o[:, 4 * r + 2]
        s2 += float(o[:, 16].sum())
        s3 += float(o[:, 17].sum())

    rank_loss = losses.sum() / N
    prec = float((ms < 0.5).mean())
    gathered = cls_fea[np.arange(N), targets].astype(np.float64)
    xent = float((lse - gathered).mean())
    side = np.sqrt(s2) + np.sqrt(s3)
    acc = float((np.argmax(x, axis=1).astype(np.int64) == targets).mean())
    total = rank_loss + xent + 0.1 * side
    prec2 = max(prec, acc)
    return np.array([total, prec2], np.float32)


def kernel(**inputs):
    global LAST_EXEC_NS
    cls_fea = np.ascontiguousarray(np.asarray(inputs["cls_fea"], np.float32))
    l2 = np.asarray(inputs["l2_side"], np.float32)
    l3 = np.asarray(inputs["l3_side"], np.float32)
    l4 = np.asarray(inputs["l4_side"], np.float32)
    x = np.asarray(inputs["input_fea"], np.float32)
    targets = np.asarray(inputs["targets"]).astype(np.int64)

    in_maps = _make_in_maps(cls_fea, l2, l3, l4, x)
    nc = _build_program()
    trace = os.environ.get("KERNEL_TRACE", "0") == "1"
    res = run_bass_kernel_spmd(nc, in_maps, list(range(NCORES)), trace=trace)
    LAST_EXEC_NS = res.exec_time_ns
    return _postprocess(res.results, cls_fea, x, targets)


# revision 8
# speedup vs baseline: 8.5305x; 8.5305x over previous
import os
import numpy as np
from concourse import bass, tile
from concourse import mybir
from concourse.bass_utils import run_bass_kernel_spmd
import bass_rust as _bass_rust

dt = mybir.dt
Alu = mybir.AluOpType
Act = mybir.ActivationFunctionType

N = 4096
F = 512
C = 751
SIDE = 1024
NCORES = 8
RPC = N // NCORES      # 512 rows per core
NT = RPC // 128        # 4 row tiles per core
NEGBIG = np.float32(-1e30)

LAST_EXEC_NS = None


def _build_program(reps=1):
    nc = bass.Bass()
    xtp_d = [nc.dram_tensor(f"xtp{k}", [128, N + RPC], dt.float32r,
                            kind="ExternalInput") for k in range(4)]
    rk2_d = nc.dram_tensor("rk2", [2, N + RPC], dt.float32, kind="ExternalInput")
    ms_d = nc.dram_tensor("mstrip", [128, 1800], dt.float32, kind="ExternalInput")
    cls_d = nc.dram_tensor("cls", [RPC, C], dt.float32, kind="ExternalInput")
    l2_d = nc.dram_tensor("l2", [RPC, SIDE], dt.float32, kind="ExternalInput")
    l3_d = nc.dram_tensor("l3", [RPC, SIDE], dt.float32, kind="ExternalInput")
    l4_d = nc.dram_tensor("l4", [RPC, SIDE], dt.float32, kind="ExternalInput")
    out_d = nc.dram_tensor("out", [128, 18], dt.float32, kind="ExternalOutput")

    with tile.TileContext(nc) as tc:
        with tc.tile_pool(name="sb", bufs=1) as sb, \
             tc.tile_pool(name="ps", bufs=8, space="PSUM") as ps:
            xtp_t = [sb.tile([128, N + RPC], dt.float32r, name=f"xtp_t{k}")
                     for k in range(4)]
            rk2_t = sb.tile([2, N + RPC], dt.float32)
            ms_t = sb.tile([128, 1800], dt.float32)
            cls_t = [sb.tile([128, C], dt.float32, name=f"cls_t{r}")
                     for r in range(NT)]
            l2_t = [sb.tile([128, SIDE], dt.float32, name=f"l2_t{r}")
                    for r in range(NT)]
            l3_t = [sb.tile([128, SIDE], dt.float32, name=f"l3_t{r}")
                    for r in range(NT)]
            l4_t = [sb.tile([128, SIDE], dt.float32, name=f"l4_t{r}")
                    for r in range(NT)]
            out_t = sb.tile([128, 18], dt.float32)
            a2 = sb.tile([128, NT], dt.float32)
            a3 = sb.tile([128, NT], dt.float32)

            tmpA = sb.tile([128, 512], dt.float32)
            tmpB = sb.tile([128, 512], dt.float32)
            cand = sb.tile([128, 64], dt.float32)
            pos8 = sb.tile([128, 8], dt.float32)
            neg8 = sb.tile([128, 8], dt.float32)
            pos8r = sb.tile([128, 8], dt.float32)
            cmp = sb.tile([128, 8], dt.float32)
            m_t = sb.tile([128, 1], dt.float32)
            clampP = sb.tile([128, 8], dt.float32)
            pP = sb.tile([128, 8], dt.float32)
            clampN = sb.tile([128, 8], dt.float32)
            nN = sb.tile([128, 8], dt.float32)
            n0e = sb.tile([128, 1], dt.float32)
            rec = sb.tile([128, 1], dt.float32)
            dlt = sb.tile([128, 8], dt.float32)
            rat = sb.tile([128, 8], dt.float32)
            E = sb.tile([128, 8], dt.float32)
            w0 = sb.tile([128, 8], dt.float32)
            ind = sb.tile([128, 8], dt.float32)
            diff = sb.tile([128, 8], dt.float32)
            t1 = sb.tile([128, 8], dt.float32)
            t2 = sb.tile([128, 8], dt.float32)
            t3 = sb.tile([128, 8], dt.float32)
            t5 = sb.tile([128, 8], dt.float32)
            l8 = sb.tile([128, 8], dt.float32)
            negmax = sb.tile([128, 1], dt.float32)
            scr = sb.tile([128, C], dt.float32)
            se = sb.tile([128, 1], dt.float32)
            lse = sb.tile([128, 1], dt.float32)
            sd2 = sb.tile([128, SIDE], dt.float32)
            sq2 = sb.tile([128, SIDE], dt.float32)
            sd3 = sb.tile([128, SIDE], dt.float32)
            sq3 = sb.tile([128, SIDE], dt.float32)

            kt = ms_t[:, 1792:1800]

            for _ in range(reps):
                for k in range(4):
                    nc.gpsimd.dma_start(xtp_t[k][:], xtp_d[k][:])
                nc.gpsimd.dma_start(rk2_t[:], rk2_d[:])
                nc.gpsimd.dma_start(ms_t[:], ms_d[:])
                for r in range(NT):
                    nc.gpsimd.dma_start(cls_t[r][:],
                                        cls_d[128 * r:128 * r + 128, :])
                    nc.gpsimd.dma_start(l2_t[r][:],
                                        l2_d[128 * r:128 * r + 128, :])
                    nc.gpsimd.dma_start(l3_t[r][:],
                                        l3_d[128 * r:128 * r + 128, :])
                    nc.gpsimd.dma_start(l4_t[r][:],
                                        l4_d[128 * r:128 * r + 128, :])

                for r in range(NT):
                    lT = slice(N + 128 * r, N + 128 * r + 128)
                    for cb in range(8):
                        cS = slice(512 * cb, 512 * cb + 512)
                        p = ps.tile([128, 512], dt.float32, name="p")
                        for k in range(4):
                            nc.tensor.matmul(p[:], xtp_t[k][:, lT],
                                             xtp_t[k][:, cS],
                                             start=(k == 0), stop=False)
                        nc.tensor.matmul(p[:], rk2_t[:, lT], rk2_t[:, cS],
                                         start=False, stop=True)
                        if cb == 0:
                            nc.vector.tensor_tensor(
                                tmpA[:], p[:],
                                ms_t[:, 384 - 128 * r:896 - 128 * r], Alu.add)
                            nc.vector.max(pos8[:], tmpA[:])
                            nc.vector.tensor_tensor(
                                tmpB[:], p[:],
                                ms_t[:, 1280 - 128 * r:1792 - 128 * r],
                                Alu.add)
                            nc.vector.max(cand[:, 0:8], tmpB[:])
                        else:
                            nc.vector.max(cand[:, 8 * cb:8 * cb + 8], p[:])

                    nc.vector.max(neg8[:], cand[:])
                    nc.vector.tensor_scalar_add(pos8r[:], pos8[:, 7::-1], 0.0)
                    nc.vector.tensor_tensor(cmp[:], neg8[:], pos8r[:],
                                            Alu.is_gt)
                    nc.vector.tensor_reduce(m_t[:], cmp[:],
                                            mybir.AxisListType.X, Alu.add)
                    nc.vector.tensor_scalar(clampP[:], pos8r[:], -1.0, 1e-12,
                                            Alu.mult, Alu.max)
                    nc.scalar.activation(pP[:], clampP[:], Act.Sqrt)
                    nc.vector.tensor_scalar(clampN[:], neg8[:], -1.0, 1e-12,
                                            Alu.mult, Alu.max)
                    nc.scalar.activation(nN[:], clampN[:], Act.Sqrt)
                    nc.vector.tensor_scalar_add(n0e[:], nN[:, 0:1], 1e-12)
                    nc.vector.reciprocal(rec[:], n0e[:])
                    nc.vector.tensor_scalar(dlt[:], nN[:], nN[:, 0:1], -1.0,
                                            Alu.subtract, Alu.mult)
                    nc.vector.tensor_scalar(rat[:], dlt[:], rec[:], 0.0,
                                            Alu.mult, Alu.add)
                    nc.scalar.activation(E[:], rat[:], Act.Exp)
                    nc.vector.tensor_scalar(w0[:], kt, m_t[:], -1.0,
                                            Alu.subtract, Alu.mult)
                    nc.vector.tensor_scalar(ind[:], w0[:], 0.0, 1.0,
                                            Alu.max, Alu.min)
                    nc.vector.tensor_tensor(diff[:], pP[:], nN[:],
                                            Alu.subtract)
                    nc.vector.tensor_tensor(t1[:], E[:], diff[:], Alu.mult)
                    nc.vector.tensor_tensor(t2[:], t1[:], w0[:], Alu.mult)
                    nc.vector.tensor_tensor(t3[:], t2[:], ind[:], Alu.mult)
                    nc.vector.tensor_scalar(t5[:], ind[:], 0.5, 0.0,
                                            Alu.mult, Alu.add)
                    nc.vector.tensor_tensor(l8[:], t3[:], t5[:], Alu.add)
                    nc.vector.tensor_reduce(out_t[:, 4 * r:4 * r + 1], l8[:],
                                            mybir.AxisListType.X, Alu.add)
                    nc.vector.tensor_scalar_add(out_t[:, 4 * r + 1:4 * r + 2],
                                                m_t[:], 0.0)

                    nc.vector.tensor_reduce(negmax[:], cls_t[r][:],
                                            mybir.AxisListType.X, Alu.max,
                                            negate=True)
                    nc.scalar.activation(scr[:], cls_t[r][:], Act.Exp,
                                         bias=negmax[:], scale=1.0,
                                         accum_out=se[:])
                    nc.scalar.activation(lse[:], se[:], Act.Ln)
                    nc.vector.tensor_tensor(out_t[:, 4 * r + 2:4 * r + 3],
                                            lse[:], negmax[:], Alu.subtract)

                    nc.vector.tensor_tensor(sd2[:], l4_t[r][:], l2_t[r][:],
                                            Alu.subtract)
                    nc.scalar.activation(sq2[:], sd2[:], Act.Square,
                                         accum_out=a2[:, r:r + 1])
                    nc.vector.tensor_tensor(sd3[:], l4_t[r][:], l3_t[r][:],
                                            Alu.subtract)
                    nc.scalar.activation(sq3[:], sd3[:], Act.Square,
                                         accum_out=a3[:, r:r + 1])

                nc.vector.tensor_reduce(out_t[:, 16:17], a2[:],
                                        mybir.AxisListType.X, Alu.add)
                nc.vector.tensor_reduce(out_t[:, 17:18], a3[:],
                                        mybir.AxisListType.X, Alu.add)
                nc.sync.dma_start(out_d[:], out_t[:])

    _bass_rust.move_matmul_waits_to_ldweights(nc.m)
    _bass_rust.generate_event_semaphores(nc)
    return nc


def _build_mstrip():
    K0 = np.full((128, 128), NEGBIG, np.float32)
    D0 = np.zeros((128, 128), np.float32)
    for b in range(16):
        K0[8 * b:8 * b + 8, 8 * b:8 * b + 8] = 0.0
        D0[8 * b:8 * b + 8, 8 * b:8 * b + 8] = NEGBIG
    kstrip = np.full((128, 896), NEGBIG, np.float32)
    kstrip[:, 384:512] = K0
    drop = np.zeros((128, 896), np.float32)
    drop[:, 384:512] = D0
    kv = np.tile(np.arange(8, dtype=np.float32), (128, 1))
    return np.ascontiguousarray(np.concatenate([kstrip, drop, kv], axis=1))


def _make_in_maps(cls_fea, l2, l3, l4, x):
    sq = (x.astype(np.float64) ** 2).sum(1).astype(np.float32)
    xT = np.ascontiguousarray(x.T)
    mstrip = _build_mstrip()

    in_maps = []
    for c in range(NCORES):
        R0 = RPC * c
        perm = np.concatenate([np.arange(R0, R0 + RPC),
                               np.arange(0, R0),
                               np.arange(R0 + RPC, N)])
        xt_perm = xT[:, perm]
        im = {}
        for k in range(4):
            im[f"xtp{k}"] = np.ascontiguousarray(np.concatenate(
                [xt_perm[128 * k:128 * k + 128, :],
                 2.0 * xt_perm[128 * k:128 * k + 128, 0:RPC]], axis=1))
        sqp = sq[perm]
        rk2 = np.empty((2, N + RPC), np.float32)
        rk2[0, 0:N] = -sqp
        rk2[0, N:] = 1.0
        rk2[1, 0:N] = 1.0
        rk2[1, N:] = -sq[R0:R0 + RPC]
        im["rk2"] = rk2
        im["mstrip"] = mstrip
        im["cls"] = np.ascontiguousarray(cls_fea[R0:R0 + RPC])
        im["l2"] = np.ascontiguousarray(l2[R0:R0 + RPC])
        im["l3"] = np.ascontiguousarray(l3[R0:R0 + RPC])
        im["l4"] = np.ascontiguousarray(l4[R0:R0 + RPC])
        in_maps.append(im)
    return in_maps


def _postprocess(results, cls_fea, x, targets):
    losses = np.empty(N, np.float64)
    ms = np.empty(N, np.float64)
    lse = np.empty(N, np.float64)
    s2 = 0.0
    s3 = 0.0
    for c in range(NCORES):
        o = np.asarray(results[c]["out"], np.float64)
        for r in range(NT):
            rows = slice(RPC * c + 128 * r, RPC * c + 128 * r + 128)
            losses[rows] = o[:, 4 * r]
            ms[rows] = o[:, 4 * r + 1]
            lse[rows] = o[:, 4 * r + 2]
        s2 += float(o[:, 16].sum())
        s3 += float(o[:, 17].sum())

    rank_loss = losses.sum() / N
    prec = float((ms < 0.5).mean())
    gathered = cls_fea[np.arange(N), targets].astype(np.float64)
    xent = float((lse - gathered).mean())
    side = np.sqrt(s2) + np.sqrt(s3)
    acc = float((np.argmax(x, axis=1).astype(np.int64) == targets).mean())
    total = rank_loss + xent + 0.1 * side
    prec2 = max(prec, acc)
    return np.array([total, prec2], np.float32)


def kernel(**inputs):
    global LAST_EXEC_NS
    cls_fea = np.ascontiguousarray(np.asarray(inputs["cls_fea"], np.float32))
    l2 = np.asarray(inputs["l2_side"], np.float32)
    l3 = np.asarray(inputs["l3_side"], np.float32)
    l4 = np.asarray(inputs["l4_side"], np.float32)
    x = np.asarray(inputs["input_fea"], np.float32)
    targets = np.asarray(inputs["targets"]).astype(np.int64)

    in_maps = _make_in_maps(cls_fea, l2, l3, l4, x)
    nc = _build_program()
    trace = os.environ.get("KERNEL_TRACE", "0") == "1"
    res = run_bass_kernel_spmd(nc, in_maps, list(range(NCORES)), trace=trace)
    LAST_EXEC_NS = res.exec_time_ns
    return _postprocess(res.results, cls_fea, x, targets)
